# revision 16
# baseline (speedup 1.0000x reference)
"""Alpha-filter (keras_spiking AlphaCell) Trainium2 Bass kernel.

Math: per (batch b, feature k) the reference runs the 2-state recurrence
    x_t = A_k x_{t-1} + B_k u_t,   y_t = x_t[1]
with A_k = e*[[1-a, -a/tau],[dt, 1+a]], a = dt/tau, e = exp(-a).
A_k has a defective double eigenvalue e (A = e(I+N), N nilpotent), so the
recurrence reduces to two CHAINED first-order scans (scan2 consumes scan1's
output directly — no intermediate tensor):

    s_t   = e * s_{t-1} + u_t          s_0   = L/(1-e)      (L = initial_level)
    eta_t = e * eta_{t-1} + s_{t-1}    eta_0 = L/(1-e)^2
    y_t   = [e*a*(1-e)] * eta_t + [(1-e) - e*a] * s_t

Each scan is one DVE tensor_tensor_scan over a [128 features, T] tile
(time on the free dim); the per-feature multiplier stream is a stride-0
broadcast AP of a [128,1] column (verified exact on HW).  Input tiles
arrive in [time, features] layout (contiguous DMA) and are transposed on
the PE into PSUM; scan1 reads PSUM directly.  The combine is two in-place
ScalarE pre-scales followed by PE transpose-back pairs accumulating in
PSUM; ScalarE copies y to SBUF for the contiguous store.

Sharding: data-parallel over batch, 8 batches per core x 8 cores.
"""

import sys

for _p in ("/opt/trn_rl_repo",):
    if _p not in sys.path:
        sys.path.insert(0, _p)

from contextlib import ExitStack

import numpy as np

import concourse.bacc as bacc
import concourse.bass as bass
import concourse.tile as tile
from concourse import mybir
from concourse.bass_utils import run_bass_kernel_spmd

DT = 0.001
B, T, K = 64, 1024, 512
N_CORES = 8
B_LOC = B // N_CORES  # 8 batches per core
P = 128
KC = K // P   # 4 feature chunks of 128
TCH = T // P  # 8 time chunks of 128

F32 = mybir.dt.float32
MULT = mybir.AluOpType.mult
ADD = mybir.AluOpType.add


def _bcast(col_ap, n):
    """[P,1] AP -> [P,n] stride-0 free-dim broadcast AP."""
    return bass.AP(tensor=col_ap.tensor, offset=col_ap.offset, ap=[col_ap.ap[0], [0, n]])


def build_nc():
    nc = bacc.Bacc(None, target_bir_lowering=False)

    x = nc.dram_tensor("x", [B_LOC, T, K], F32, kind="ExternalInput")
    # cols[c] = [e, s0, eta0, c_eta, c_s] per feature chunk
    cols = nc.dram_tensor("cols", [KC, 5, P], F32, kind="ExternalInput")
    ident = nc.dram_tensor("ident", [P, P], F32, kind="ExternalInput")
    y = nc.dram_tensor("y", [B_LOC, T, K], F32, kind="ExternalOutput")

    with tile.TileContext(nc) as tc, ExitStack() as ctx:
        singles = ctx.enter_context(tc.tile_pool(name="singles", bufs=1))
        inpool = ctx.enter_context(tc.tile_pool(name="inpool", bufs=2))
        outpool = ctx.enter_context(tc.tile_pool(name="outpool", bufs=2))
        epool = ctx.enter_context(tc.tile_pool(name="epool", bufs=2 * KC))
        psum_u = ctx.enter_context(tc.tile_pool(name="psum_u", bufs=2, space="PSUM"))
        psum_y = ctx.enter_context(tc.tile_pool(name="psum_y", bufs=4, space="PSUM"))

        # ---- one-time constant loads -----------------------------------
        def load_col(idx, name):
            t = singles.tile([P, KC], F32, tag=name)
            nc.scalar.dma_start(out=t[:], in_=cols.rearrange("c s p -> p c s")[:, :, idx])
            return t

        ident_t = singles.tile([P, P], F32)
        nc.sync.dma_start(out=ident_t[:], in_=ident[:])
        e_col = load_col(0, "e_col")
        s0_col = load_col(1, "s0_col")
        eta0_col = load_col(2, "eta0_col")
        ceta_col = load_col(3, "ceta_col")
        cs_col = load_col(4, "cs_col")

        # PE warm-up during the initial DMA window: HAM needs ~3.4us of
        # activity before the PE clock doubles; burn it on scratch transposes.
        scratch = singles.tile([P, P], F32)
        nc.gpsimd.memset(scratch[:], 0.0)
        warm = psum_y.tile([P, K], F32, name="warm", tag="yp")
        for i in range(8):
            nc.tensor.transpose(warm[:, 0:P], scratch[:], ident_t[:])

        # static double-buffered s tiles: col 0 = s0 written once per tile
        s_static = [
            [
                singles.tile(
                    [P, T + 1], F32, tag=f"s_{c}_{par}", name=f"s_{c}_{par}"
                )
                for par in range(2)
            ]
            for c in range(KC)
        ]
        for c in range(KC):
            for par in range(2):
                nc.scalar.copy(s_static[c][par][:, 0:1], s0_col[:, c : c + 1])

        # ---- main loop over local batches ------------------------------
        for b in range(B_LOC):
            par = b % 2
            # First and last batch run their scans as two chained halves:
            # the head's first scan starts after half the input is staged,
            # and the tail's output phase overlaps the last half-scans.
            split = b == B_LOC - 1
            # staged input: in_stage[p, tch, k] = x[b, tch*128+p, k]
            # split DMA per t-chunk group so transposes can start early
            in_stage = inpool.tile([P, TCH, K], F32)
            xv = x[b].rearrange("(a p) k -> p a k", p=P)
            for h in range(0, TCH, 4):
                nc.sync.dma_start(
                    out=in_stage[:, h : h + 4, :],
                    in_=xv[:, h : h + 4, :],
                )

            s_tiles = []
            eta_tiles = []
            for c in range(KC):
                # transpose u into [128 features, T] (PSUM), time along free
                uT = psum_u.tile([P, T], F32)
                for t in range(TCH):
                    nc.tensor.transpose(
                        uT[:, t * P : (t + 1) * P],
                        in_stage[:, t, c * P : (c + 1) * P],
                        ident_t[:],
                    )

                s_full = s_static[c][par]
                eta = epool.tile([P, T], F32)
                ecb = e_col[:, c : c + 1]
                if not split:
                    nc.vector.tensor_tensor_scan(
                        out=s_full[:, 1 : T + 1],
                        data0=_bcast(ecb, T),
                        data1=uT[:],
                        initial=s0_col[:, c : c + 1],
                        op0=MULT,
                        op1=ADD,
                    )
                    nc.vector.tensor_tensor_scan(
                        out=eta[:],
                        data0=_bcast(ecb, T),
                        data1=s_full[:, 0:T],
                        initial=eta0_col[:, c : c + 1],
                        op0=MULT,
                        op1=ADD,
                    )
                    nc.scalar.mul(eta[:], eta[:], ceta_col[:, c : c + 1])
                    shat = epool.tile([P, T], F32, tag="shat")
                    nc.scalar.mul(
                        shat[:], s_full[:, 1 : T + 1], cs_col[:, c : c + 1]
                    )
                else:
                    H = T // 2
                    nc.vector.tensor_tensor_scan(
                        out=s_full[:, 1 : H + 1],
                        data0=_bcast(ecb, H),
                        data1=uT[:, 0:H],
                        initial=s0_col[:, c : c + 1],
                        op0=MULT,
                        op1=ADD,
                    )
                    nc.vector.tensor_tensor_scan(
                        out=s_full[:, H + 1 : T + 1],
                        data0=_bcast(ecb, H),
                        data1=uT[:, H:T],
                        initial=s_full[:, H : H + 1],
                        op0=MULT,
                        op1=ADD,
                    )
                    nc.vector.tensor_tensor_scan(
                        out=eta[:, 0:H],
                        data0=_bcast(ecb, H),
                        data1=s_full[:, 0:H],
                        initial=eta0_col[:, c : c + 1],
                        op0=MULT,
                        op1=ADD,
                    )
                    # copy the half-boundary eta so the h1 pre-scale (WAR on
                    # eta[:,0:H]) cannot block the h2 scan's initial read
                    etacol = epool.tile([P, 1], F32, tag="etacol")
                    nc.scalar.copy(etacol[:], eta[:, H - 1 : H])
                    nc.vector.tensor_tensor_scan(
                        out=eta[:, H:T],
                        data0=_bcast(ecb, H),
                        data1=s_full[:, H:T],
                        initial=etacol[:],
                        op0=MULT,
                        op1=ADD,
                    )
                    shat = epool.tile([P, T], F32, tag="shat")
                    for lo, hi in ((0, H), (H, T)):
                        nc.scalar.mul(
                            eta[:, lo:hi], eta[:, lo:hi], ceta_col[:, c : c + 1]
                        )
                        nc.scalar.mul(
                            shat[:, lo:hi],
                            s_full[:, lo + 1 : hi + 1],
                            cs_col[:, c : c + 1],
                        )
                s_tiles.append(shat)
                eta_tiles.append(eta)

            # transpose back; the combine is the PSUM accumulation of the
            # two pre-scaled transposes: y[t,k] = eta_hat[k,t] + s_hat[k,t]
            out_stage = outpool.tile([P, TCH, K], F32)
            yv = y[b].rearrange("(a p) k -> p a k", p=P)
            for t in range(TCH):
                yp = psum_y.tile([P, K], F32)
                for c in range(KC):
                    nc.tensor.matmul(
                        yp[:, c * P : (c + 1) * P],
                        eta_tiles[c][:, t * P : (t + 1) * P],
                        ident_t[:],
                        is_transpose=True,
                        start=True,
                        stop=False,
                    )
                    nc.tensor.matmul(
                        yp[:, c * P : (c + 1) * P],
                        s_tiles[c][:, t * P : (t + 1) * P],
                        ident_t[:],
                        is_transpose=True,
                        start=False,
                        stop=True,
                    )
                nc.scalar.copy(out_stage[:, t, :], yp[:])
                if b == B_LOC - 1:
                    nc.sync.dma_start(out=yv[:, t, :], in_=out_stage[:, t, :])
                elif t % 2 == 1:
                    h = t // 2
                    nc.sync.dma_start(
                        out=yv[:, h * 2 : (h + 1) * 2, :],
                        in_=out_stage[:, h * 2 : (h + 1) * 2, :],
                    )

    nc.compile()
    return nc


_CACHE = {}
PROFILE = False
LAST_RESULT = None


def _host_constants(initial_level, tau):
    tau_c = np.maximum(tau.astype(np.float64), 1e-8)
    a = DT / tau_c
    e = np.exp(-a)
    em1 = 1.0 - e
    ea = e * a
    s0 = initial_level.astype(np.float64) / em1
    eta0 = initial_level.astype(np.float64) / (em1 * em1)
    c_eta = ea * em1
    c_s = em1 - ea
    cols = np.stack(
        [
            e.astype(np.float32).reshape(KC, P),
            s0.astype(np.float32).reshape(KC, P),
            eta0.astype(np.float32).reshape(KC, P),
            c_eta.astype(np.float32).reshape(KC, P),
            c_s.astype(np.float32).reshape(KC, P),
        ],
        axis=1,
    )  # [KC, 5, P]
    ident = np.eye(P, dtype=np.float32)
    return cols, ident


def kernel(inputs, initial_level, tau):
    global LAST_RESULT
    inputs = np.ascontiguousarray(np.asarray(inputs, dtype=np.float32))
    initial_level = np.asarray(initial_level, dtype=np.float32)
    tau = np.asarray(tau, dtype=np.float32)
    assert inputs.shape == (B, T, K), inputs.shape

    cols, ident = _host_constants(initial_level, tau)

    if "nc" not in _CACHE:
        _CACHE["nc"] = build_nc()
    nc = _CACHE["nc"]

    in_maps = [
        {
            "x": inputs[i * B_LOC : (i + 1) * B_LOC],
            "cols": cols,
            "ident": ident,
        }
        for i in range(N_CORES)
    ]
    res = run_bass_kernel_spmd(nc, in_maps, list(range(N_CORES)), trace=PROFILE)
    LAST_RESULT = res
    return np.concatenate([r["y"] for r in res.results], axis=0)
